# revision 26
# baseline (speedup 1.0000x reference)
"""Trainium2 Bass kernel for nn_ConstraintsModule.

Reference math:
    m = preds[:, atoms]                                   # [B, N]
    body_rev[b,c,j] = pos_body[c,j] + m[b,j]*(neg_body-pos_body)[c,j]
    body_min[b,c]   = 1 - max_j body_rev[b,c,j]
    lb[b,n] = max_c body_min[b,c]*pos_head[c,n]
    ub[b,n] = 1 - max_c body_min[b,c]*neg_head[c,n]
    updated = clamp(m, min(lb,ub), max(lb,ub))
    out = preds with columns `atoms` replaced by updated

Device pipeline (per core, one SPMD program):
  * body_min[b,c] = min( min_{j in pos(c)} m[b,j],
                         min_{j in neg(c)} (1-m[b,j]) )
    -> host packs per-constraint slots [pos m values | neg (1-m) values]
    (bf16, padded to even width with 1.0); GpSimd does a stride-2
    pairwise min (one op per DMA chunk), then DVE strided min-reduces
    each uniform-width region to body_min.
  * head phase: TensorE transposes body_min (slots onto partitions) and
    multiplies with a per-core one-hot scatter matrix (input data), which
    lands each body_min in an [atom-group, round] grid in PSUM; one DVE
    strided max-reduce produces lb / ubm for all atom groups at once.
  * output: [lb | ubm] in bf16 (exact: all values are bf16-rounded
    already). The host merges split atom groups (bins larger than the
    round count R=2 span several groups), forms ub = 1-ubm, clamps the
    fp32 m, and scatters into preds. Only the bf16 rounding of m and
    (1-m) at pack time perturbs the result: rel err ~4e-3 vs the 2e-2
    tolerance.

Sharding: whole constraints (grouped by head atom) are dealt to the
8 cores balancing slot count (=128 each) and packed width; the program
is shared (slot widths are the per-index max across cores), only packed
data and scatter matrices differ per core.
"""

import sys
from contextlib import ExitStack

import numpy as np

if "/opt/trn_rl_repo" not in sys.path:
    sys.path.insert(0, "/opt/trn_rl_repo")

import ml_dtypes

import concourse.bacc as bacc
import concourse.tile as tile
from concourse import masks, mybir
from concourse.bass_utils import run_bass_kernel_spmd

BF16 = ml_dtypes.bfloat16

B = 128
C = 1024
N = 512
NCORES = 8
S = 128           # constraint slots per core
WROUND = 4        # slot widths rounded up to this (even: stride-2 fold safe)
R = 2             # bin rounds per atom group (bigger bins split, host merges)
SLOT_BOUNDS = (0, 16, 34, 50, 64, 82, 96, 112, 128)   # G DMA chunk boundaries (slots)
TSPLIT = 64       # transpose/matmul piece boundary (PE base partition 0/32/64)
# width-rank -> slot permutation: a small mid-width block leads (fast first
# DMA chunk + immediate DVE work), the widest block follows, narrow ranks
# trail (small last chunk on the critical tail)
_RANK2SLOT = tuple(
    list(range(16, 64)) + list(range(0, 16)) + list(range(64, 128))
)

# Set by test.py to profile; the grading path leaves these alone.
_TRACE = False
_LAST_RESULTS = None

_PROGRAM_CACHE: dict = {}


def _roundup(x: int, mult: int) -> int:
    return ((x + mult - 1) // mult) * mult


def _build_program(widths, na_pad):
    """widths: tuple of S per-slot packed widths (shared across cores)."""
    key = (widths, na_pad)
    if key in _PROGRAM_CACHE:
        return _PROGRAM_CACHE[key]

    dt = mybir.dt
    col_off = np.concatenate([[0], np.cumsum(widths)]).astype(int)
    total_cols = int(col_off[-1])
    PC = 2 * R * na_pad            # pos block then neg block, R cols per group

    nc = bacc.Bacc(
        "TRN2", target_bir_lowering=False, debug=False, enable_partition_id=False
    )
    c_ds = []
    for i in range(len(SLOT_BOUNDS) - 1):
        s0, s1 = SLOT_BOUNDS[i], SLOT_BOUNDS[i + 1]
        c_ds.append(
            nc.dram_tensor(
                f"c{i}", [B, int(col_off[s1] - col_off[s0])], dt.bfloat16,
                kind="ExternalInput",
            )
        )
    p_d = nc.dram_tensor("pmat", [S, PC], dt.bfloat16, kind="ExternalInput")
    out_d = nc.dram_tensor("lbubm", [B, 2 * na_pad], dt.bfloat16, kind="ExternalOutput")

    with ExitStack() as ctx:
        tc = ctx.enter_context(tile.TileContext(nc))
        pool = ctx.enter_context(tc.tile_pool(name="main", bufs=1))
        psum = ctx.enter_context(tc.tile_pool(name="psum", bufs=1, space="PSUM"))

        g_sb = pool.tile([B, total_cols], dt.bfloat16, tag="g_sb")
        bmin = pool.tile([B, S], dt.bfloat16, tag="bmin")
        p_sb = pool.tile([S, PC], dt.bfloat16, tag="p_sb")
        iden = pool.tile([128, 128], dt.bfloat16, tag="iden")
        bminT = pool.tile([S, B], dt.bfloat16, tag="bminT")
        ps_pos = psum.tile([B, R * na_pad], dt.float32, tag="ps_pos")
        ps_neg = psum.tile([B, R * na_pad], dt.float32, tag="ps_neg")

        # GpSimd builds the transpose identity first; the scatter matrix
        # DMA follows so its SDMA traffic stays clear of the first G chunks
        # (it is only needed by the piece-1 matmul, several us later)
        masks.make_identity(nc, iden[:])
        nc.gpsimd.dma_start(p_sb[:], p_d.ap())

        def emit_reduces(s0, s1):
            # one strided min-reduce per uniform-width run inside the chunk
            r0 = s0
            while r0 < s1:
                w = widths[r0]
                r1 = r0
                while r1 < s1 and widths[r1] == w:
                    r1 += 1
                g3 = g_sb[:, int(col_off[r0]) : int(col_off[r1])].rearrange(
                    "p (c k) -> p c k", k=w
                )
                nc.vector.tensor_reduce(
                    bmin[:, r0:r1], g3,
                    axis=mybir.AxisListType.X, op=mybir.AluOpType.min,
                )
                r0 = r1

        def emit_transpose(t0, t1, on_act=False):
            tp = psum.tile([t1 - t0, B], dt.bfloat16, tag=f"tp{t0}")
            nc.tensor.transpose(tp[:], bmin[:, t0:t1], iden[:])
            if on_act:
                # ACT is idle mid-stream; keep the busy DVE on body reduces
                nc.scalar.copy(bminT[t0:t1, :], tp[:])
            else:
                nc.vector.tensor_copy(bminT[t0:t1, :], tp[:])

        def emit_matmuls(t0, t1, first, last):
            nc.tensor.matmul(
                ps_pos[:], bminT[t0:t1, :], p_sb[t0:t1, 0 : R * na_pad],
                start=first, stop=last,
            )
            nc.tensor.matmul(
                ps_neg[:], bminT[t0:t1, :], p_sb[t0:t1, R * na_pad : PC],
                start=first, stop=last,
            )

        # Scatter steps (transpose slots onto partitions + one-hot matmul
        # into the [atom-group, round] PSUM grid) are interleaved into the
        # chunk stream — DVE and PE execute in emission order, so piece 1
        # (slots 0:64) runs under the shadow of later chunk DMAs and only
        # the final 16 slots' transpose sits on the critical tail.
        dma_engines = [nc.sync, nc.scalar]
        for i in range(len(SLOT_BOUNDS) - 1):
            s0, s1 = SLOT_BOUNDS[i], SLOT_BOUNDS[i + 1]
            o0, o1 = int(col_off[s0]), int(col_off[s1])
            dma_engines[i % 2].dma_start(g_sb[:, o0:o1], c_ds[i].ap())
            emit_reduces(s0, s1)
            if s1 == TSPLIT:
                emit_transpose(0, TSPLIT)
                emit_matmuls(0, TSPLIT, True, False)
            elif s1 == 96:
                emit_transpose(TSPLIT, 96, on_act=True)
        emit_transpose(96, S)
        emit_matmuls(TSPLIT, S, False, True)

        # [p, (group r)] max over r -> [lb | ubm]; two halves so the first
        # DMA's descriptor generation overlaps the second reduce.
        lbubm = pool.tile([B, 2 * na_pad], dt.bfloat16, tag="lbubm")
        for k, ps in enumerate((ps_pos, ps_neg)):
            nc.vector.tensor_reduce(
                lbubm[:, k * na_pad : (k + 1) * na_pad],
                ps[:].rearrange("p (n r) -> p n r", r=R),
                axis=mybir.AxisListType.X, op=mybir.AluOpType.max,
            )
            dma_engines[k].dma_start(
                out_d.ap()[:, k * na_pad : (k + 1) * na_pad],
                lbubm[:, k * na_pad : (k + 1) * na_pad],
            )

    nc.compile()
    _PROGRAM_CACHE[key] = nc
    return nc


def kernel(preds, pos_head, neg_head, pos_body, neg_body, atoms):
    global _LAST_RESULTS
    preds = np.ascontiguousarray(np.asarray(preds, dtype=np.float32))
    pos_head = np.asarray(pos_head)
    neg_head = np.asarray(neg_head)
    pos_body = np.asarray(pos_body)
    neg_body = np.asarray(neg_body)
    atoms_np = np.asarray(atoms).astype(np.int64)

    m = np.ascontiguousarray(preds[:, atoms_np].astype(np.float32))  # [B, N]
    # packed value source: [bf16(m) | bf16(1-m) | 1.0 pad]
    m2 = np.concatenate(
        [
            m.astype(BF16),
            (np.float32(1.0) - m).astype(BF16),
            np.ones((B, 1), BF16),
        ],
        axis=1,
    )
    PAD = 2 * N

    pb = pos_body != 0
    nb_ = neg_body != 0
    body_js = [(np.nonzero(pb[c])[0], np.nonzero(nb_[c])[0]) for c in range(C)]
    w_pad = np.array(
        [max(_roundup(len(jp) + len(jn), WROUND), WROUND) for jp, jn in body_js]
    )

    ph_atom = pos_head.argmax(1)
    ph_has = pos_head.max(1) > 0
    nh_atom = neg_head.argmax(1)
    nh_has = neg_head.max(1) > 0
    pos_bins = [[] for _ in range(N)]
    neg_bins = [[] for _ in range(N)]
    for c in np.nonzero(ph_has)[0]:
        pos_bins[ph_atom[c]].append(int(c))
    for c in np.nonzero(nh_has)[0]:
        neg_bins[nh_atom[c]].append(int(c))
    atom_cons = [pos_bins[a] + neg_bins[a] for a in range(N)]
    used_atoms = [a for a in range(N) if atom_cons[a]]
    # groups per atom: R rounds each; bins larger than R span several groups
    n_groups = {
        a: max(-(-len(pos_bins[a]) // R), -(-len(neg_bins[a]) // R), 1)
        for a in used_atoms
    }

    # Deal atoms (whole constraint groups) to cores: exact slot-count
    # balance first (<= S slots), then total packed width, then group count.
    order = sorted(
        used_atoms,
        key=lambda a: (-len(atom_cons[a]), -int(sum(w_pad[c] for c in atom_cons[a]))),
    )
    core_cnt = [0] * NCORES
    core_w = [0] * NCORES
    core_g = [0] * NCORES
    core_atoms = [[] for _ in range(NCORES)]
    for a in order:
        k = len(atom_cons[a])
        wa = int(sum(w_pad[c] for c in atom_cons[a]))
        cands = [i for i in range(NCORES) if core_cnt[i] + k <= S]
        assert cands, "atom dealing infeasible"
        i = min(cands, key=lambda i: (core_w[i], core_g[i], core_cnt[i]))
        core_cnt[i] += k
        core_w[i] += wa
        core_g[i] += n_groups[a]
        core_atoms[i].append(a)

    na_pad = _roundup(max(core_g), 4)

    # Per-core width rank: constraints sorted by padded width desc; the
    # k-th widest constraint of every core shares slot _RANK2SLOT[k].
    core_ranked = []
    for i in range(NCORES):
        cons = [c for a in core_atoms[i] for c in atom_cons[a]]
        cons.sort(key=lambda c: (-w_pad[c], c))
        core_ranked.append(cons)

    # Shared per-slot widths: max across cores (dummy slots width WROUND).
    widths = np.full(S, WROUND, np.int64)
    for cons in core_ranked:
        for r, c in enumerate(cons):
            j = _RANK2SLOT[r]
            widths[j] = max(widths[j], w_pad[c])
    widths = tuple(int(x) for x in widths)
    col_off = np.concatenate([[0], np.cumsum(widths)]).astype(int)
    total_cols = int(col_off[-1])

    nc = _build_program(widths, na_pad)

    in_maps = []
    scatter = []  # per core: list of (atom, [pos group cols], [neg group cols])
    PC = 2 * R * na_pad
    for core in range(NCORES):
        cons = core_ranked[core]
        slot_of = {c: _RANK2SLOT[r] for r, c in enumerate(cons)}
        g_idx = np.full(total_cols, PAD, np.int64)
        for c in cons:
            jp, jn = body_js[c]
            o = int(col_off[slot_of[c]])
            g_idx[o : o + jp.size] = jp
            g_idx[o + jp.size : o + jp.size + jn.size] = N + jn
        g_vals = np.ascontiguousarray(m2[:, g_idx])

        pmat = np.zeros((S, PC), BF16)
        core_scatter = []
        g0 = 0
        for a in core_atoms[core]:
            ng = n_groups[a]
            for r, c in enumerate(pos_bins[a]):
                pmat[slot_of[c], (g0 + r // R) * R + (r % R)] = 1.0
            for r, c in enumerate(neg_bins[a]):
                pmat[slot_of[c], R * na_pad + (g0 + r // R) * R + (r % R)] = 1.0
            npg = -(-len(pos_bins[a]) // R)
            nng = -(-len(neg_bins[a]) // R)
            core_scatter.append(
                (a, list(range(g0, g0 + npg)), list(range(g0, g0 + nng)))
            )
            g0 += ng
        assert g0 <= na_pad

        im = {"pmat": pmat}
        for i in range(len(SLOT_BOUNDS) - 1):
            s0, s1 = SLOT_BOUNDS[i], SLOT_BOUNDS[i + 1]
            im[f"c{i}"] = np.ascontiguousarray(
                g_vals[:, int(col_off[s0]) : int(col_off[s1])]
            )
        in_maps.append(im)
        scatter.append(core_scatter)

    res = run_bass_kernel_spmd(
        nc, in_maps, core_ids=list(range(NCORES)), trace=_TRACE
    )
    _LAST_RESULTS = res

    # Host: merge split groups, ub = 1 - ubm, clamp fp32 m, scatter.
    out = preds.copy()
    for core in range(NCORES):
        lbubm = np.asarray(res.results[core]["lbubm"]).astype(np.float32)
        for a, pg, ngr in scatter[core]:
            lb = lbubm[:, pg].max(1) if pg else np.float32(0.0)
            ubm = lbubm[:, [na_pad + g for g in ngr]].max(1) if ngr else np.float32(0.0)
            ub = np.float32(1.0) - ubm
            lo = np.minimum(lb, ub)
            hi = np.maximum(lb, ub)
            ma = m[:, a]
            out[:, atoms_np[a]] = np.maximum(lo, np.minimum(hi, ma))
    return out


# revision 27
# speedup vs baseline: 1.0634x; 1.0634x over previous
"""Trainium2 Bass kernel for nn_ConstraintsModule.

Reference math:
    m = preds[:, atoms]                                   # [B, N]
    body_rev[b,c,j] = pos_body[c,j] + m[b,j]*(neg_body-pos_body)[c,j]
    body_min[b,c]   = 1 - max_j body_rev[b,c,j]
    lb[b,n] = max_c body_min[b,c]*pos_head[c,n]
    ub[b,n] = 1 - max_c body_min[b,c]*neg_head[c,n]
    updated = clamp(m, min(lb,ub), max(lb,ub))
    out = preds with columns `atoms` replaced by updated

Device pipeline (per core, one SPMD program):
  * body_min[b,c] = min( min_{j in pos(c)} m[b,j],
                         min_{j in neg(c)} (1-m[b,j]) )
    -> host packs per-constraint slots [pos m values | neg (1-m) values]
    (bf16, padded to even width with 1.0); GpSimd does a stride-2
    pairwise min (one op per DMA chunk), then DVE strided min-reduces
    each uniform-width region to body_min.
  * head phase: TensorE transposes body_min (slots onto partitions) and
    multiplies with a per-core one-hot scatter matrix (input data), which
    lands each body_min in an [atom-group, round] grid in PSUM; one DVE
    strided max-reduce produces lb / ubm for all atom groups at once.
  * output: [lb | ubm] in bf16 (exact: all values are bf16-rounded
    already). The host merges split atom groups (bins larger than the
    round count R=2 span several groups), forms ub = 1-ubm, clamps the
    fp32 m, and scatters into preds. Only the bf16 rounding of m and
    (1-m) at pack time perturbs the result: rel err ~4e-3 vs the 2e-2
    tolerance.

Sharding: whole constraints (grouped by head atom) are dealt to the
8 cores balancing slot count (=128 each) and packed width; the program
is shared (slot widths are the per-index max across cores), only packed
data and scatter matrices differ per core.
"""

import sys
from contextlib import ExitStack

import numpy as np

if "/opt/trn_rl_repo" not in sys.path:
    sys.path.insert(0, "/opt/trn_rl_repo")

import ml_dtypes

import concourse.bacc as bacc
import concourse.tile as tile
from concourse import masks, mybir
from concourse.bass_utils import run_bass_kernel_spmd

BF16 = ml_dtypes.bfloat16

B = 128
C = 1024
N = 512
NCORES = 8
S = 128           # constraint slots per core
WROUND = 4        # slot widths rounded up to this (even: stride-2 fold safe)
R = 2             # bin rounds per atom group (bigger bins split, host merges)
SLOT_BOUNDS = (0, 16, 40, 64, 96, 114, 128)   # G DMA chunk boundaries (slots)
TSPLIT = 64       # transpose/matmul piece boundary (PE base partition 0/32/64)
# width-rank -> slot permutation: a small mid-width block leads (fast first
# DMA chunk + immediate DVE work), the widest block follows, narrow ranks
# trail (small last chunk on the critical tail)
_RANK2SLOT = tuple(
    list(range(16, 64)) + list(range(0, 16)) + list(range(64, 128))
)

# Set by test.py to profile; the grading path leaves these alone.
_TRACE = False
_LAST_RESULTS = None

_PROGRAM_CACHE: dict = {}


def _roundup(x: int, mult: int) -> int:
    return ((x + mult - 1) // mult) * mult


def _build_program(widths, na_pad):
    """widths: tuple of S per-slot packed widths (shared across cores)."""
    key = (widths, na_pad)
    if key in _PROGRAM_CACHE:
        return _PROGRAM_CACHE[key]

    dt = mybir.dt
    col_off = np.concatenate([[0], np.cumsum(widths)]).astype(int)
    total_cols = int(col_off[-1])
    PC = 2 * R * na_pad            # pos block then neg block, R cols per group

    nc = bacc.Bacc(
        "TRN2", target_bir_lowering=False, debug=False, enable_partition_id=False
    )
    c_ds = []
    for i in range(len(SLOT_BOUNDS) - 1):
        s0, s1 = SLOT_BOUNDS[i], SLOT_BOUNDS[i + 1]
        c_ds.append(
            nc.dram_tensor(
                f"c{i}", [B, int(col_off[s1] - col_off[s0])], dt.bfloat16,
                kind="ExternalInput",
            )
        )
    p_d = nc.dram_tensor("pmat", [S, PC], dt.bfloat16, kind="ExternalInput")
    out_d = nc.dram_tensor("lbubm", [B, 2 * na_pad], dt.bfloat16, kind="ExternalOutput")

    with ExitStack() as ctx:
        tc = ctx.enter_context(tile.TileContext(nc))
        pool = ctx.enter_context(tc.tile_pool(name="main", bufs=1))
        psum = ctx.enter_context(tc.tile_pool(name="psum", bufs=1, space="PSUM"))

        g_sb = pool.tile([B, total_cols], dt.bfloat16, tag="g_sb")
        bmin = pool.tile([B, S], dt.bfloat16, tag="bmin")
        p_sb = pool.tile([S, PC], dt.bfloat16, tag="p_sb")
        iden = pool.tile([128, 128], dt.bfloat16, tag="iden")
        bminT = pool.tile([S, B], dt.bfloat16, tag="bminT")
        ps_pos = psum.tile([B, R * na_pad], dt.float32, tag="ps_pos")
        ps_neg = psum.tile([B, R * na_pad], dt.float32, tag="ps_neg")

        # GpSimd builds the transpose identity first; the scatter matrix
        # DMA follows so its SDMA traffic stays clear of the first G chunks
        # (it is only needed by the piece-1 matmul, several us later)
        masks.make_identity(nc, iden[:])
        nc.gpsimd.dma_start(p_sb[:], p_d.ap())

        def emit_reduces(s0, s1):
            # one strided min-reduce per uniform-width run inside the chunk
            r0 = s0
            while r0 < s1:
                w = widths[r0]
                r1 = r0
                while r1 < s1 and widths[r1] == w:
                    r1 += 1
                g3 = g_sb[:, int(col_off[r0]) : int(col_off[r1])].rearrange(
                    "p (c k) -> p c k", k=w
                )
                nc.vector.tensor_reduce(
                    bmin[:, r0:r1], g3,
                    axis=mybir.AxisListType.X, op=mybir.AluOpType.min,
                )
                r0 = r1

        def emit_transpose(t0, t1, on_act=False):
            tp = psum.tile([t1 - t0, B], dt.bfloat16, tag=f"tp{t0}")
            nc.tensor.transpose(tp[:], bmin[:, t0:t1], iden[:])
            if on_act:
                # ACT is idle mid-stream; keep the busy DVE on body reduces
                nc.scalar.copy(bminT[t0:t1, :], tp[:])
            else:
                nc.vector.tensor_copy(bminT[t0:t1, :], tp[:])

        def emit_matmuls(t0, t1, first, last):
            nc.tensor.matmul(
                ps_pos[:], bminT[t0:t1, :], p_sb[t0:t1, 0 : R * na_pad],
                start=first, stop=last,
            )
            nc.tensor.matmul(
                ps_neg[:], bminT[t0:t1, :], p_sb[t0:t1, R * na_pad : PC],
                start=first, stop=last,
            )

        # Scatter steps (transpose slots onto partitions + one-hot matmul
        # into the [atom-group, round] PSUM grid) are interleaved into the
        # chunk stream — DVE and PE execute in emission order, so piece 1
        # (slots 0:64) runs under the shadow of later chunk DMAs and only
        # the final 16 slots' transpose sits on the critical tail.
        dma_engines = [nc.sync, nc.scalar]
        for i in range(len(SLOT_BOUNDS) - 1):
            s0, s1 = SLOT_BOUNDS[i], SLOT_BOUNDS[i + 1]
            o0, o1 = int(col_off[s0]), int(col_off[s1])
            dma_engines[i % 2].dma_start(g_sb[:, o0:o1], c_ds[i].ap())
            emit_reduces(s0, s1)
            if s1 == TSPLIT:
                emit_transpose(0, TSPLIT)
                emit_matmuls(0, TSPLIT, True, False)
            elif s1 == 96:
                emit_transpose(TSPLIT, 96, on_act=True)
        emit_transpose(96, S)
        emit_matmuls(TSPLIT, S, False, True)

        # [p, (group r)] max over r -> [lb | ubm]; two halves so the first
        # DMA's descriptor generation overlaps the second reduce.
        lbubm = pool.tile([B, 2 * na_pad], dt.bfloat16, tag="lbubm")
        for k, ps in enumerate((ps_pos, ps_neg)):
            nc.vector.tensor_reduce(
                lbubm[:, k * na_pad : (k + 1) * na_pad],
                ps[:].rearrange("p (n r) -> p n r", r=R),
                axis=mybir.AxisListType.X, op=mybir.AluOpType.max,
            )
            dma_engines[k].dma_start(
                out_d.ap()[:, k * na_pad : (k + 1) * na_pad],
                lbubm[:, k * na_pad : (k + 1) * na_pad],
            )

    nc.compile()
    _PROGRAM_CACHE[key] = nc
    return nc


def kernel(preds, pos_head, neg_head, pos_body, neg_body, atoms):
    global _LAST_RESULTS
    preds = np.ascontiguousarray(np.asarray(preds, dtype=np.float32))
    pos_head = np.asarray(pos_head)
    neg_head = np.asarray(neg_head)
    pos_body = np.asarray(pos_body)
    neg_body = np.asarray(neg_body)
    atoms_np = np.asarray(atoms).astype(np.int64)

    m = np.ascontiguousarray(preds[:, atoms_np].astype(np.float32))  # [B, N]
    # packed value source: [bf16(m) | bf16(1-m) | 1.0 pad]
    m2 = np.concatenate(
        [
            m.astype(BF16),
            (np.float32(1.0) - m).astype(BF16),
            np.ones((B, 1), BF16),
        ],
        axis=1,
    )
    PAD = 2 * N

    pb = pos_body != 0
    nb_ = neg_body != 0
    body_js = [(np.nonzero(pb[c])[0], np.nonzero(nb_[c])[0]) for c in range(C)]
    w_pad = np.array(
        [max(_roundup(len(jp) + len(jn), WROUND), WROUND) for jp, jn in body_js]
    )

    ph_atom = pos_head.argmax(1)
    ph_has = pos_head.max(1) > 0
    nh_atom = neg_head.argmax(1)
    nh_has = neg_head.max(1) > 0
    pos_bins = [[] for _ in range(N)]
    neg_bins = [[] for _ in range(N)]
    for c in np.nonzero(ph_has)[0]:
        pos_bins[ph_atom[c]].append(int(c))
    for c in np.nonzero(nh_has)[0]:
        neg_bins[nh_atom[c]].append(int(c))
    atom_cons = [pos_bins[a] + neg_bins[a] for a in range(N)]
    used_atoms = [a for a in range(N) if atom_cons[a]]
    # groups per atom: R rounds each; bins larger than R span several groups
    n_groups = {
        a: max(-(-len(pos_bins[a]) // R), -(-len(neg_bins[a]) // R), 1)
        for a in used_atoms
    }

    # Deal atoms (whole constraint groups) to cores: exact slot-count
    # balance first (<= S slots), then total packed width, then group count.
    order = sorted(
        used_atoms,
        key=lambda a: (-len(atom_cons[a]), -int(sum(w_pad[c] for c in atom_cons[a]))),
    )
    core_cnt = [0] * NCORES
    core_w = [0] * NCORES
    core_g = [0] * NCORES
    core_atoms = [[] for _ in range(NCORES)]
    for a in order:
        k = len(atom_cons[a])
        wa = int(sum(w_pad[c] for c in atom_cons[a]))
        cands = [i for i in range(NCORES) if core_cnt[i] + k <= S]
        assert cands, "atom dealing infeasible"
        i = min(cands, key=lambda i: (core_w[i], core_g[i], core_cnt[i]))
        core_cnt[i] += k
        core_w[i] += wa
        core_g[i] += n_groups[a]
        core_atoms[i].append(a)

    na_pad = _roundup(max(core_g), 4)

    # Per-core width rank: constraints sorted by padded width desc; the
    # k-th widest constraint of every core shares slot _RANK2SLOT[k].
    core_ranked = []
    for i in range(NCORES):
        cons = [c for a in core_atoms[i] for c in atom_cons[a]]
        cons.sort(key=lambda c: (-w_pad[c], c))
        core_ranked.append(cons)

    # Shared per-slot widths: max across cores (dummy slots width WROUND).
    widths = np.full(S, WROUND, np.int64)
    for cons in core_ranked:
        for r, c in enumerate(cons):
            j = _RANK2SLOT[r]
            widths[j] = max(widths[j], w_pad[c])
    widths = tuple(int(x) for x in widths)
    col_off = np.concatenate([[0], np.cumsum(widths)]).astype(int)
    total_cols = int(col_off[-1])

    nc = _build_program(widths, na_pad)

    in_maps = []
    scatter = []  # per core: list of (atom, [pos group cols], [neg group cols])
    PC = 2 * R * na_pad
    for core in range(NCORES):
        cons = core_ranked[core]
        slot_of = {c: _RANK2SLOT[r] for r, c in enumerate(cons)}
        g_idx = np.full(total_cols, PAD, np.int64)
        for c in cons:
            jp, jn = body_js[c]
            o = int(col_off[slot_of[c]])
            g_idx[o : o + jp.size] = jp
            g_idx[o + jp.size : o + jp.size + jn.size] = N + jn
        g_vals = np.ascontiguousarray(m2[:, g_idx])

        pmat = np.zeros((S, PC), BF16)
        core_scatter = []
        g0 = 0
        for a in core_atoms[core]:
            ng = n_groups[a]
            for r, c in enumerate(pos_bins[a]):
                pmat[slot_of[c], (g0 + r // R) * R + (r % R)] = 1.0
            for r, c in enumerate(neg_bins[a]):
                pmat[slot_of[c], R * na_pad + (g0 + r // R) * R + (r % R)] = 1.0
            npg = -(-len(pos_bins[a]) // R)
            nng = -(-len(neg_bins[a]) // R)
            core_scatter.append(
                (a, list(range(g0, g0 + npg)), list(range(g0, g0 + nng)))
            )
            g0 += ng
        assert g0 <= na_pad

        im = {"pmat": pmat}
        for i in range(len(SLOT_BOUNDS) - 1):
            s0, s1 = SLOT_BOUNDS[i], SLOT_BOUNDS[i + 1]
            im[f"c{i}"] = np.ascontiguousarray(
                g_vals[:, int(col_off[s0]) : int(col_off[s1])]
            )
        in_maps.append(im)
        scatter.append(core_scatter)

    res = run_bass_kernel_spmd(
        nc, in_maps, core_ids=list(range(NCORES)), trace=_TRACE
    )
    _LAST_RESULTS = res

    # Host: merge split groups, ub = 1 - ubm, clamp fp32 m, scatter.
    out = preds.copy()
    for core in range(NCORES):
        lbubm = np.asarray(res.results[core]["lbubm"]).astype(np.float32)
        for a, pg, ngr in scatter[core]:
            lb = lbubm[:, pg].max(1) if pg else np.float32(0.0)
            ubm = lbubm[:, [na_pad + g for g in ngr]].max(1) if ngr else np.float32(0.0)
            ub = np.float32(1.0) - ubm
            lo = np.minimum(lb, ub)
            hi = np.maximum(lb, ub)
            ma = m[:, a]
            out[:, atoms_np[a]] = np.maximum(lo, np.minimum(hi, ma))
    return out
